# revision 8
# baseline (speedup 1.0000x reference)
"""Trainium2 Bass kernel for a 3-layer GraphSAGE GNN (mean aggregation).

Strategy (8 NeuronCores, dst-node sharding):
  - Nodes are padded to Ntot = 8 * Nl and sharded contiguously: core c owns
    nodes [c*Nl, (c+1)*Nl).
  - Edges are sorted by dst and assigned to the core owning dst. Each core
    processes its dst nodes in tiles of 128; each tile's incoming edges are
    padded to a uniform B blocks of 128 edges.
  - Per layer, each core gathers h[src] rows from a replicated row-layout h
    table in DRAM via one batched indirect DMA per node tile, builds a
    {0,1} selection matrix S on-device (iota + is_equal), and computes the
    segment sum as a PE matmul accumulation:  aggT[f, n] += M_b^T @ S_b.
  - Everything downstream runs in transposed orientation [feat, node] so no
    transposes are needed in the hot loop:  h_nextT = relu(W_l^T @ (aggT *
    inv_deg) + W_r^T @ h_ownT + b_l).  Only the row-layout write-back for the
    next layer's gather needs one PE transpose per tile.
  - After the encoder and layers 0..L-2, the row-layout shard is AllGathered
    across the 8 cores into the next replicated h table.
"""

import math

import numpy as np

P = 128  # partitions / tile size
C = 8  # cores

_CACHE = {}
LAST_RESULT = None
LAST_RUN_S = None


def _build(cfg):
    """Build (nc, meta) for the given static config."""
    import concourse.bacc as bacc
    import concourse.tile as tile
    from concourse import bass, mybir
    from concourse._compat import axon_active
    from concourse.masks import make_identity

    f32 = mybir.dt.float32
    i32 = mybir.dt.int32

    DIN, H, DOUT, L = cfg["DIN"], cfg["H"], cfg["DOUT"], cfg["L"]
    Nl, B, T = cfg["Nl"], cfg["B"], cfg["T"]
    Ntot = Nl * C

    nc = bacc.Bacc(
        "TRN2",
        target_bir_lowering=False,
        debug=False,
        enable_asserts=not axon_active(),
        num_devices=C,
    )

    # ---- I/O ----
    xT_t = nc.dram_tensor("xT", [DIN, Nl], f32, kind="ExternalInput")
    sidx_t = nc.dram_tensor("sidx", [P, T * B], i32, kind="ExternalInput")
    dstv_t = nc.dram_tensor("dstv", [P, T * B], f32, kind="ExternalInput")
    idb_t = nc.dram_tensor("invdegb", [P, Nl], f32, kind="ExternalInput")
    we1_t = nc.dram_tensor("W_enc1", [DIN, H], f32, kind="ExternalInput")
    be1_t = nc.dram_tensor("b_enc1", [H], f32, kind="ExternalInput")
    we2_t = nc.dram_tensor("W_enc2", [H, H], f32, kind="ExternalInput")
    be2_t = nc.dram_tensor("b_enc2", [H], f32, kind="ExternalInput")
    wl_t = nc.dram_tensor("W_l", [L, H, H], f32, kind="ExternalInput")
    bl_t = nc.dram_tensor("b_l", [L, H], f32, kind="ExternalInput")
    wr_t = nc.dram_tensor("W_r", [L, H, H], f32, kind="ExternalInput")
    wd1_t = nc.dram_tensor("W_dec1", [H, H], f32, kind="ExternalInput")
    bd1_t = nc.dram_tensor("b_dec1", [H], f32, kind="ExternalInput")
    wd2_t = nc.dram_tensor("W_dec2", [H, DOUT], f32, kind="ExternalInput")
    bd2_t = nc.dram_tensor("b_dec2", [DOUT], f32, kind="ExternalInput")
    hout_t = nc.dram_tensor("h_out", [Nl, H], f32, kind="ExternalOutput")
    xdout_t = nc.dram_tensor("xd_out", [DOUT, Nl], f32, kind="ExternalOutput")

    rg = [list(range(C))]
    relu = mybir.ActivationFunctionType.Relu

    with tile.TileContext(nc) as tc:
        from contextlib import ExitStack

        with ExitStack() as ctx:
            const = ctx.enter_context(tc.tile_pool(name="const", bufs=1))
            dram = ctx.enter_context(tc.tile_pool(name="dram", bufs=1, space="DRAM"))
            work = ctx.enter_context(tc.tile_pool(name="work", bufs=3))
            psum = ctx.enter_context(tc.tile_pool(name="psum", bufs=2, space="PSUM"))

            # ---- DRAM scratch: per-layer replicated h tables + own shards ----
            h_full = [
                dram.tile([Ntot, H], f32, name=f"h_full{l}", addr_space="Shared")
                for l in range(L)
            ]
            h_shard = [dram.tile([Nl, H], f32, name=f"h_shard{l}") for l in range(L)]

            # ---- resident SBUF ----
            sidx_sb = const.tile([P, T * B], i32)
            nc.sync.dma_start(sidx_sb[:], sidx_t.ap())
            dstv_sb = const.tile([P, T * B], f32)
            nc.sync.dma_start(dstv_sb[:], dstv_t.ap())
            hT_sb = const.tile([H, Nl], f32)

            niota_i = work.tile([P, B * P], i32, tag="M")
            nc.gpsimd.iota(niota_i[:], pattern=[[0, B], [1, P]], base=0,
                           channel_multiplier=0)
            niota_f = const.tile([P, B * P], f32)
            nc.vector.tensor_copy(niota_f[:], niota_i[:])

            identity = const.tile([P, P], f32)
            make_identity(nc, identity[:])

            def _load(shape, t, apsel=None):
                s = const.tile(shape, f32, name=f"c_{t.name}")
                nc.sync.dma_start(s[:], t.ap() if apsel is None else apsel)
                return s

            we1_sb = _load([DIN, H], we1_t)
            we2_sb = _load([H, H], we2_t)
            be1_sb = _load([H, 1], be1_t, be1_t.ap()[:, None])
            be2_sb = _load([H, 1], be2_t, be2_t.ap()[:, None])
            wl_sb = [_load([H, H], wl_t, wl_t.ap()[l]) for l in range(L)]
            wr_sb = [_load([H, H], wr_t, wr_t.ap()[l]) for l in range(L)]
            bl_sb = [_load([H, 1], bl_t, bl_t.ap()[l, :, None]) for l in range(L)]
            wd1_sb = _load([H, H], wd1_t)
            bd1_sb = _load([H, 1], bd1_t, bd1_t.ap()[:, None])
            wd2_sb = _load([H, DOUT], wd2_t)
            bd2_sb = _load([DOUT, 1], bd2_t, bd2_t.ap()[:, None])

            # ---- encoder (own shard only) ----
            for t in range(T):
                sl = slice(t * P, (t + 1) * P)
                xT_tile = work.tile([DIN, P], f32, tag="xT")
                nc.sync.dma_start(xT_tile[:], xT_t.ap()[:, sl])
                ps1 = psum.tile([H, P], f32, tag="ps_a")
                nc.tensor.matmul(ps1[:], lhsT=we1_sb[:], rhs=xT_tile[:],
                                 start=True, stop=True)
                e1 = work.tile([H, P], f32, tag="e1")
                nc.scalar.activation(e1[:], ps1[:], relu, bias=be1_sb[:, 0:1])
                ps2 = psum.tile([H, P], f32, tag="ps_agg")
                nc.tensor.matmul(ps2[:], lhsT=we2_sb[:], rhs=e1[:],
                                 start=True, stop=True)
                nc.scalar.activation(hT_sb[:, sl], ps2[:], relu,
                                     bias=be2_sb[:, 0:1])
                psr = psum.tile([P, H], f32, tag="ps_row")
                nc.tensor.transpose(psr[:], hT_sb[:, sl], identity[:])
                row = work.tile([P, H], f32, tag="row")
                nc.vector.tensor_copy(row[:], psr[:])
                nc.sync.dma_start(h_shard[0][sl, :], row[:])

            nc.gpsimd.collective_compute(
                "AllGather", mybir.AluOpType.bypass, replica_groups=rg,
                ins=[h_shard[0].opt()], outs=[h_full[0].opt()],
            )

            # ---- SAGE layers ----
            for l in range(L):
                for t in range(T):
                    sl = slice(t * P, (t + 1) * P)
                    slb = slice(t * B, (t + 1) * B)
                    M = work.tile([P, B * P], f32, tag="M")
                    for b in range(B):
                        # HW indirect DMA consumes one index per partition and
                        # gathers out-free-size contiguous elements, so issue
                        # one call per 128-edge block ([P,1] idx -> [P,H] out).
                        nc.gpsimd.indirect_dma_start(
                            out=M[:, b * P:(b + 1) * P], out_offset=None,
                            in_=h_full[l][:, :],
                            in_offset=bass.IndirectOffsetOnAxis(
                                ap=sidx_sb[:, t * B + b:t * B + b + 1], axis=0),
                        )
                    S = work.tile([P, B * P], f32, tag="S")
                    nc.vector.tensor_tensor(
                        out=S[:].rearrange("p (b n) -> p b n", n=P),
                        in0=dstv_sb[:, slb, None].to_broadcast([P, B, P]),
                        in1=niota_f[:].rearrange("p (b n) -> p b n", n=P),
                        op=mybir.AluOpType.is_equal,
                    )
                    pagg = psum.tile([H, P], f32, tag="ps_agg")
                    for b in range(B):
                        bs = slice(b * P, (b + 1) * P)
                        nc.tensor.matmul(pagg[:], lhsT=M[:, bs], rhs=S[:, bs],
                                         start=(b == 0), stop=(b == B - 1))
                    idb_tile = work.tile([P, P], f32, tag="idb")
                    nc.sync.dma_start(idb_tile[:], idb_t.ap()[:, sl])
                    aggT = work.tile([H, P], f32, tag="aggT")
                    nc.vector.tensor_tensor(out=aggT[:], in0=pagg[:],
                                            in1=idb_tile[:],
                                            op=mybir.AluOpType.mult)
                    pout = psum.tile([H, P], f32, tag="ps_a")
                    nc.tensor.matmul(pout[:], lhsT=wl_sb[l][:], rhs=aggT[:],
                                     start=True, stop=False)
                    nc.tensor.matmul(pout[:], lhsT=wr_sb[l][:], rhs=hT_sb[:, sl],
                                     start=False, stop=True)
                    nc.scalar.activation(hT_sb[:, sl], pout[:], relu,
                                         bias=bl_sb[l][:, 0:1])
                    psr = psum.tile([P, H], f32, tag="ps_row")
                    nc.tensor.transpose(psr[:], hT_sb[:, sl], identity[:])
                    row = work.tile([P, H], f32, tag="row")
                    nc.vector.tensor_copy(row[:], psr[:])
                    if l < L - 1:
                        nc.sync.dma_start(h_shard[l + 1][sl, :], row[:])
                    else:
                        nc.sync.dma_start(hout_t.ap()[sl, :], row[:])
                if l < L - 1:
                    nc.gpsimd.collective_compute(
                        "AllGather", mybir.AluOpType.bypass, replica_groups=rg,
                        ins=[h_shard[l + 1].opt()], outs=[h_full[l + 1].opt()],
                    )

            # ---- decoder (own shard only) ----
            for t in range(T):
                sl = slice(t * P, (t + 1) * P)
                pd = psum.tile([H, P], f32, tag="ps_a")
                nc.tensor.matmul(pd[:], lhsT=wd1_sb[:], rhs=hT_sb[:, sl],
                                 start=True, stop=True)
                dT = work.tile([H, P], f32, tag="e1")
                nc.scalar.activation(dT[:], pd[:], relu, bias=bd1_sb[:, 0:1])
                po = psum.tile([DOUT, P], f32, tag="ps_row")
                nc.tensor.matmul(po[:], lhsT=wd2_sb[:], rhs=dT[:],
                                 start=True, stop=True)
                xd = work.tile([DOUT, P], f32, tag="xd")
                nc.vector.tensor_scalar_add(xd[:], po[:], bd2_sb[:, 0:1])
                nc.sync.dma_start(xdout_t.ap()[:, sl], xd[:])

    nc.compile()
    return nc


def _prep(x, edge_index):
    """Host-side index preprocessing: shard, sort, pad. Returns per-core arrays."""
    N = x.shape[0]
    Nl = int(math.ceil(N / (C * P))) * P
    Ntot = Nl * C
    T = Nl // P

    src = edge_index[0].astype(np.int64)
    dst = edge_index[1].astype(np.int64)

    deg = np.bincount(dst, minlength=Ntot).astype(np.float32)
    inv_deg = np.where(deg > 0, 1.0 / deg, 0.0).astype(np.float32)

    order = np.argsort(dst, kind="stable")
    src_s = src[order]
    dst_s = dst[order]

    tile_of_edge = dst_s // P
    counts = np.bincount(tile_of_edge, minlength=C * T)
    B = max(1, int(np.ceil(counts.max() / P)))
    cap = B * P

    offs = np.zeros(C * T + 1, np.int64)
    np.cumsum(counts, out=offs[1:])

    # slot k within a tile's padded edge list -> device slot (p=k%P, b=k//P)
    sidx = np.zeros((C * T, cap), np.int32)
    dstv = np.full((C * T, cap), float(P + 71), np.float32)  # pad: never matches
    pos = np.arange(len(dst_s)) - offs[tile_of_edge]  # rank within tile
    flat = tile_of_edge * cap + pos
    sidx.reshape(-1)[flat] = src_s.astype(np.int32)
    dstv.reshape(-1)[flat] = (dst_s - tile_of_edge * P).astype(np.float32)

    sidx_dev = sidx.reshape(C, T, B, P).transpose(0, 3, 1, 2).reshape(C, P, T * B)
    dstv_dev = dstv.reshape(C, T, B, P).transpose(0, 3, 1, 2).reshape(C, P, T * B)

    x_pad = np.zeros((Ntot, x.shape[1]), np.float32)
    x_pad[:N] = x
    xT = np.ascontiguousarray(
        x_pad.reshape(C, Nl, x.shape[1]).transpose(0, 2, 1))

    idb = np.ascontiguousarray(
        np.broadcast_to(inv_deg.reshape(C, 1, Nl), (C, P, Nl)))

    return dict(Nl=Nl, T=T, B=B, sidx=sidx_dev, dstv=dstv_dev, xT=xT, idb=idb)


def kernel(**inputs):
    from concourse.bass_utils import run_bass_kernel_spmd

    x = np.asarray(inputs["x"], np.float32)
    edge_index = np.asarray(inputs["edge_index"])
    N, DIN = x.shape
    H = inputs["W_enc1"].shape[1]
    L = inputs["W_l"].shape[0]
    DOUT = inputs["W_dec2"].shape[1]

    pkey = (edge_index.shape[1], x.shape[0],
            int(edge_index[:, :64].astype(np.int64).sum()),
            int(edge_index[:, -64:].astype(np.int64).sum()))
    if pkey in _CACHE:
        prep = _CACHE[pkey]
    else:
        prep = _prep(x, edge_index)
        _CACHE[pkey] = prep
    Nl, T, B = prep["Nl"], prep["T"], prep["B"]

    cfg = dict(DIN=DIN, H=H, DOUT=DOUT, L=L, Nl=Nl, B=B, T=T)
    key = tuple(sorted(cfg.items()))
    if key not in _CACHE:
        _CACHE[key] = _build(cfg)
    nc = _CACHE[key]

    weights = {
        k: np.ascontiguousarray(np.asarray(inputs[k], np.float32))
        for k in ("W_enc1", "b_enc1", "W_enc2", "b_enc2", "W_l", "b_l", "W_r",
                  "W_dec1", "b_dec1", "W_dec2", "b_dec2")
    }
    in_maps = []
    for c in range(C):
        m = dict(weights)
        m["xT"] = prep["xT"][c]
        m["sidx"] = prep["sidx"][c]
        m["dstv"] = prep["dstv"][c]
        m["invdegb"] = prep["idb"][c]
        in_maps.append(m)

    import time as _time
    _t0 = _time.monotonic()
    res = run_bass_kernel_spmd(nc, in_maps, list(range(C)))
    global LAST_RESULT, LAST_RUN_S
    LAST_RESULT = res
    LAST_RUN_S = _time.monotonic() - _t0
    outs = res.results
    h = np.concatenate([outs[c]["h_out"] for c in range(C)], 0)[:N]
    xd = np.concatenate([outs[c]["xd_out"] for c in range(C)], 1).T[:N]
    return (xd, h)
